# revision 11
# baseline (speedup 1.0000x reference)
"""TRN2 Bass kernel for nn_MultiHeadSelfAttention_15822659518596.

Key algebraic fact: in the reference, softmax and V are dead code — the
output is

    out[b,i,:] = (scores[b,i].reshape(S*H)) @ W_fc.T + b_fc
    scores[b,i,j,n] = (q[b,i,n,:] . k[b,j,n,:]) / 8

which collapses into dense GEMMs without materializing the (B,S,S,H)
score tensor.  With Wq additionally folded PAST the big contraction
(v2 change — the old kernel materialized qT = Wq @ x^T, 536 MMAC/core):

    Kf_b = x_b @ Wk.T                      (S, D)   [c = n*64+kk head-major]
    M_b[c,o]  = sum_j Kf_b[j,c] * Wfc[o, j*8+n(c)] / 8      (D, OH)
    G_b[d,o]  = sum_c Wq[c,d]   * M_b[c,o]                  (D, OH)  <- tiny
    outT[o,i] = sum_d G_b[d,o]  * xT_b[d,i]                 (OH, S)

Sharding: 8 cores = (4 batches) x (2 halves of the fc output dim o).
Each core computes outT[o_half, S] for its (b, h) — no collectives
(on-chip collectives cost 5-20us floors; useless at this kernel size).

v2 (64.2us) vs baseline (75.6us): G-fold (stage-3 536->67 MMAC/core),
dual HWDGE DMA queues, bf16 output, bias instructions compiled only
when biases are nonzero (they are zero here).

v3: PE warm-up matmuls (HAM runs cold 1.2GHz until ~3.4us of sustained
activity), queue rebalance (x first-halves + wfc heads 0-4 on the
scalar queue; a combined 2KB-row wk|wq tensor + x second-halves + wfc
heads 5-7 + output on the sync queue — 1KB-row DMAs measured 173 GB/s
vs 281 for 8KB rows), kf copies on the scalar NX, output assembled to
two 0.5MB 4KB-row DMAs split across queues.
"""

import ml_dtypes
import numpy as np

import concourse.bass as bass
import concourse.tile as tile
from concourse import mybir, bacc
from concourse.bass_utils import run_bass_kernel_spmd

B, S, D, H = 4, 2048, 512, 8
DK = D // H            # 64
OH = D // 2            # 256, per-core o-half
NC = 8                 # cores
F32 = mybir.dt.float32
BF16 = mybir.dt.bfloat16
COPY = mybir.ActivationFunctionType.Identity

_CACHE = {}


def _build_program(with_bias: bool):
    """One SPMD Bass program; per-core tensors differ only in data."""
    nc = bacc.Bacc("TRN2", target_bir_lowering=False, debug=False, num_devices=NC)

    xT = nc.dram_tensor("xT", [D, S], BF16, kind="ExternalInput")          # x_b.T
    # wkq[:, :512] = Wk.T as [d, c]; wkq[:, 512:] = Wq as [c, d].
    # Combined so the sync-queue DMAs have 2KB partition rows.
    wkq = nc.dram_tensor("wkq", [D, 2 * D], BF16, kind="ExternalInput")
    wfc = nc.dram_tensor("wfc", [H, 128, 16 * OH], BF16, kind="ExternalInput")
    if with_bias:
        colsum = nc.dram_tensor("colsum", [1, H * OH], BF16, kind="ExternalInput")
        bkrow = nc.dram_tensor("bkrow", [1, D], BF16, kind="ExternalInput")
        bq_col = nc.dram_tensor("bq_col", [128, 4], BF16, kind="ExternalInput")
        bfc_row = nc.dram_tensor("bfc_row", [1, OH], F32, kind="ExternalInput")
    outT = nc.dram_tensor("outT", [OH, S], BF16, kind="ExternalOutput")

    with tile.TileContext(nc) as tc:
        with tc.tile_pool(name="xt", bufs=4) as p_xt, \
             tc.tile_pool(name="wk", bufs=4) as p_wk, \
             tc.tile_pool(name="kf", bufs=16) as p_kf, \
             tc.tile_pool(name="wf", bufs=8) as p_wf, \
             tc.tile_pool(name="m", bufs=4) as p_m, \
             tc.tile_pool(name="g", bufs=4) as p_g, \
             tc.tile_pool(name="ob", bufs=3) as p_ob, \
             tc.tile_pool(name="bias", bufs=1) as p_bias, \
             tc.tile_pool(name="psA", bufs=(2 if with_bias else 3), space="PSUM") as psA, \
             tc.tile_pool(name="psM", bufs=2, space="PSUM") as psM, \
             tc.tile_pool(name="psG", bufs=2, space="PSUM") as psG:

            # ---- PE warm-up: the HAM clock gate keeps the PE at 1.2GHz
            # until ~3.4us of sustained activity.  Dummy matmuls on a
            # memset scratch tile keep the PE busy (and then warm) while
            # the x/wk DMAs land, so stage 1 runs at 2.4GHz.
            t_wu = p_bias.tile([128, 128], BF16, tag="wu")
            nc.vector.memset(t_wu[:], 0.0)
            pw = psA.tile([128, D], F32, tag="acc")
            for _ in range(96):
                nc.tensor.matmul(pw[:, :128], t_wu[:], t_wu[:],
                                 start=True, stop=True)

            # ---- DMA assignment matches consumption order: stage 1
            # needs x first-halves + wkq by ~13us, x second-halves by
            # ~20us; stage 2 consumes wfc head-pairs (2u, 2u+1) at
            # ~27+1.7u us.  scalar queue: x halves then wfc heads 4-7;
            # sync queue: wkq then wfc heads 0-3 (and output later).
            xts = []
            for di in range(4):
                t_x = p_xt.tile([128, S], BF16, tag="xt")
                nc.scalar.dma_start(t_x[:, :S // 2],
                                    xT[di * 128:(di + 1) * 128, :S // 2])
                xts.append(t_x)
            for di in range(4):
                nc.scalar.dma_start(xts[di][:, S // 2:],
                                    xT[di * 128:(di + 1) * 128, S // 2:])
            wks, wqs, wkqs = [], [], []
            for di in range(4):
                t_kq = p_wk.tile([128, 2 * D], BF16, tag="wk")
                nc.sync.dma_start(t_kq[:], wkq[di * 128:(di + 1) * 128, :])
                wkqs.append(t_kq)
                wks.append(t_kq[:, :D])
                wqs.append(t_kq[:, D:])
            wfs = []
            for n in range(H):
                t_w = p_wf.tile([128, 16 * OH], BF16, tag="wf")
                eng = nc.sync if n < 4 else nc.scalar
                eng.dma_start(t_w[:], wfc[n][:, :])
                wfs.append(t_w)
            if with_bias:
                t_bk = p_bias.tile([1, D], BF16, tag="bk")
                nc.sync.dma_start(t_bk[:], bkrow[:])
                t_cs = p_bias.tile([1, H * OH], BF16, tag="cs")
                nc.sync.dma_start(t_cs[:], colsum[:])
                t_bq = p_bias.tile([128, 4], BF16, tag="bq")
                nc.sync.dma_start(t_bq[:], bq_col[:])
                t_bfc = p_bias.tile([1, OH], F32, tag="bfc")
                nc.sync.dma_start(t_bfc[:], bfc_row[:])
                t_ones = p_bias.tile([1, 512], BF16, tag="ones")
                nc.vector.memset(t_ones[:], 1.0)

            # ---- stage 1: Kf[j, c] (16 j-tiles), Kf = x @ Wk.T ----
            kfs = []
            for jt in range(16):
                pk = psA.tile([128, D], F32, tag="acc")
                for di in range(4):
                    nc.tensor.matmul(
                        pk[:], xts[di][:, jt * 128:(jt + 1) * 128], wks[di][:],
                        start=(di == 0), stop=(di == 3))
                t_kf = p_kf.tile([128, D], BF16, tag="kf")
                nc.scalar.activation(t_kf[:], pk[:], COPY)
                kfs.append(t_kf)

            # ---- stage 2+3 fused per head-pair u: M[c_u, o] then the
            # partial G accumulation G += Wq[c_u,:].T @ M[c_u,:].
            # bf16 matmuls support PE column-group tiling: head 2u
            # accumulates into psum[0:64] (col group 0) while head 2u+1
            # goes to psum[64:128] (col group 64) — one (128, OH) bank.
            pg0 = psG.tile([128, 512], F32, tag="pg")
            pg1 = psG.tile([128, 512], F32, tag="pg")
            pgs = [pg0[:, :OH], pg0[:, OH:], pg1[:, :OH], pg1[:, OH:]]
            # Two G accumulation groups share each bank (512 cols), so a
            # start=True would clear the sibling group's half: memset the
            # banks and run every G matmul start=False instead.
            nc.vector.memset(pg0[:], 0.0)
            nc.vector.memset(pg1[:], 0.0)
            if with_bias:
                pv = psM.tile([1, OH], F32, tag="pv")

            tms = []

            def _emit_g(u):
                if with_bias:
                    # v[o] += bq[c_u] . M[c_u, o]   (rank-1 over the block)
                    nc.tensor.matmul(pv[:], t_bq[:, u:u + 1], tms[u][:],
                                     start=(u == 0), stop=(u == 3))
                for dc in range(4):
                    nc.tensor.matmul(
                        pgs[dc][:], wqs[u][:, dc * 128:(dc + 1) * 128],
                        tms[u][:],
                        start=False, stop=(u == 3 and dc % 2 == 1),
                        skip_group_check=True)

            for u in range(4):
                n0, n1 = 2 * u, 2 * u + 1
                pm = psM.tile([128, OH], F32, tag="pm")
                # Zero the bank with DVE and run every matmul start=False:
                # per-element has_written semantics then make any schedule
                # order of the two disjoint col-group chains correct (a
                # start=True matmul would clear the WHOLE bank and race the
                # other chain, which Tile cannot see as a WAW hazard).
                nc.vector.memset(pm[:], 0.0)
                for jt in range(16):
                    nc.tensor.matmul(
                        pm[0:64, :], kfs[jt][:, n0 * 64:(n0 + 1) * 64],
                        wfs[n0][:, jt * OH:(jt + 1) * OH],
                        start=False, stop=False, tile_position=(0, 0),
                        skip_group_check=True)
                    nc.tensor.matmul(
                        pm[64:128, :], kfs[jt][:, n1 * 64:(n1 + 1) * 64],
                        wfs[n1][:, jt * OH:(jt + 1) * OH],
                        start=False, stop=(not with_bias and jt == 15),
                        tile_position=(0, 64), skip_group_check=True)
                if with_bias:
                    # exact b_qkv k-bias: M += bk[c] (x) colsum_n
                    nc.tensor.matmul(
                        pm[0:64, :], t_bk[0:1, n0 * 64:(n0 + 1) * 64],
                        t_cs[0:1, n0 * OH:(n0 + 1) * OH],
                        start=False, stop=False, tile_position=(0, 0),
                        skip_group_check=True)
                    nc.tensor.matmul(
                        pm[64:128, :], t_bk[0:1, n1 * 64:(n1 + 1) * 64],
                        t_cs[0:1, n1 * OH:(n1 + 1) * OH],
                        start=False, stop=True, tile_position=(0, 64),
                        skip_group_check=True)
                t_m = p_m.tile([128, OH], BF16, tag="m")
                nc.vector.tensor_copy(t_m[:], pm[:])
                tms.append(t_m)
                if u > 0:
                    _emit_g(u - 1)
            _emit_g(3)

            # ---- G psum -> bf16 SBUF; v += b_fc ----
            t_g0 = p_g.tile([128, 512], BF16, tag="g")
            nc.vector.tensor_copy(t_g0[:], pg0[:])
            t_g1 = p_g.tile([128, 512], BF16, tag="g")
            nc.vector.tensor_copy(t_g1[:], pg1[:])
            gs = [t_g0[:, :OH], t_g0[:, OH:], t_g1[:, :OH], t_g1[:, OH:]]
            if with_bias:
                t_vb = p_bias.tile([1, OH], BF16, tag="vb")
                v_f = p_bias.tile([1, OH], F32, tag="vf")
                nc.vector.tensor_add(v_f[:], pv[:], t_bfc[:])
                nc.vector.tensor_copy(t_vb[:], v_f[:])

            # ---- stage 4: outT[o, i] = sum_d G[d,o] * xT[d,i] (+ v[o]) ----
            for oc in range(2):
                for ic in range(4):
                    po = psA.tile([128, 512], F32, tag="acc")
                    if with_bias:
                        nc.tensor.matmul(
                            po[:], t_vb[0:1, oc * 128:(oc + 1) * 128],
                            t_ones[0:1, :], start=True, stop=False)
                    for dc in range(4):
                        nc.tensor.matmul(
                            po[:], gs[dc][:, oc * 128:(oc + 1) * 128],
                            xts[dc][:, ic * 512:(ic + 1) * 512],
                            start=(not with_bias and dc == 0), stop=(dc == 3))
                    t_o = p_ob.tile([128, 512], BF16, tag="ob")
                    nc.vector.tensor_copy(t_o[:], po[:])
                    eng = nc.scalar if (oc * 4 + ic) % 2 == 0 else nc.sync
                    eng.dma_start(
                        outT[oc * 128:(oc + 1) * 128, ic * 512:(ic + 1) * 512],
                        t_o[:])
    nc.compile()
    return nc


def _prep_inputs(x, W_qkv, b_qkv, W_fc, b_fc):
    """Host-side sharding/layout prep. O(bytes) only — no GEMM work."""
    x = np.ascontiguousarray(x, dtype=np.float32)
    W_qkv = np.asarray(W_qkv, dtype=np.float32)
    b_qkv = np.asarray(b_qkv, dtype=np.float32)
    W_fc = np.asarray(W_fc, dtype=np.float32)
    b_fc = np.asarray(b_fc, dtype=np.float32)
    with_bias = bool(np.any(b_qkv) or np.any(b_fc))

    wq = W_qkv.reshape(H, 3, DK, D)  # [n, {q,k,v}, kk, d]
    wq_cd = wq[:, 0].reshape(D, D)                        # [c, d]
    wkT = wq[:, 1].reshape(D, D).T                        # [d, c]
    wkq = np.ascontiguousarray(
        np.concatenate([wkT, wq_cd], axis=1)).astype(ml_dtypes.bfloat16)
    bq = b_qkv.reshape(H, 3, DK)
    bq_c = np.ascontiguousarray(bq[:, 0].reshape(D))      # c-order
    bk_c = np.ascontiguousarray(bq[:, 1].reshape(D))
    bq_col = np.ascontiguousarray(bq_c.reshape(4, 128).T).astype(ml_dtypes.bfloat16)
    bkrow = bk_c.reshape(1, D).astype(ml_dtypes.bfloat16)

    Wfc_s = W_fc * (1.0 / 8.0)
    # per o-half h: [n, jj, t, o] layout, plus per-head column sums
    wfc_h, cs_h, bfc_h = [], [], []
    for h in range(2):
        A = Wfc_s[h * OH:(h + 1) * OH, :]                  # (256, 16384)
        arr = np.ascontiguousarray(A.T).reshape(S, H, OH).transpose(1, 0, 2)  # [n,j,o]
        cs = np.ascontiguousarray(arr.sum(axis=1)).reshape(1, H * OH)
        arr2 = np.ascontiguousarray(
            arr.reshape(H, 16, 128, OH).transpose(0, 2, 1, 3)  # [n, jj, t, o]
        ).reshape(H, 128, 16 * OH).astype(ml_dtypes.bfloat16)
        wfc_h.append(arr2)
        cs_h.append(cs.astype(ml_dtypes.bfloat16))
        bfc_h.append(np.ascontiguousarray(b_fc[h * OH:(h + 1) * OH].reshape(1, OH)))

    xT_b = [np.ascontiguousarray(x[b].T).astype(ml_dtypes.bfloat16) for b in range(B)]

    in_maps = []
    for c in range(NC):
        b, h = c // 2, c % 2
        m = {
            "xT": xT_b[b],
            "wkq": wkq,
            "wfc": wfc_h[h],
        }
        if with_bias:
            m.update({
                "colsum": cs_h[h],
                "bkrow": bkrow,
                "bq_col": bq_col,
                "bfc_row": bfc_h[h],
            })
        in_maps.append(m)
    return in_maps, with_bias


def _run(in_maps, with_bias, trace=False, **kw):
    key = ("nc", with_bias)
    if key not in _CACHE:
        _CACHE[key] = _build_program(with_bias)
    return run_bass_kernel_spmd(
        _CACHE[key], in_maps, core_ids=list(range(NC)), trace=trace, **kw)


def _assemble(results):
    out = np.empty((B, S, D), dtype=np.float32)
    for c in range(NC):
        b, h = c // 2, c % 2
        out[b, :, h * OH:(h + 1) * OH] = results[c]["outT"].T.astype(np.float32)
    return out


def kernel(x, W_qkv, b_qkv, W_fc, b_fc):
    in_maps, with_bias = _prep_inputs(x, W_qkv, b_qkv, W_fc, b_fc)
    res = _run(in_maps, with_bias, trace=False)
    return _assemble(res.results)


def kernel_traced(x, W_qkv, b_qkv, W_fc, b_fc):
    """Like kernel() but returns (out, BassKernelResults) with NTFF trace."""
    import os
    os.environ.setdefault("BASS_PERFETTO_PROFILE_ALL_CORES", "1")
    _install_ntff_hook_shim()
    in_maps, with_bias = _prep_inputs(x, W_qkv, b_qkv, W_fc, b_fc)
    res = _run(in_maps, with_bias, trace=True)
    return _assemble(res.results), res


def _install_ntff_hook_shim():
    """The agent image's antenv lacks axon_hooks; provide it so
    run_bass_kernel_spmd(trace=True) can reach the NTFF profiler."""
    import sys, types
    if "antenv.axon_hooks" in sys.modules:
        return
    try:
        from trn_agent_boot.trn_boot import _ntff_profile_via_ctypes
    except ImportError:
        return
    mod = types.ModuleType("antenv.axon_hooks")
    _hook = [None]
    mod.set_axon_ntff_profile_hook = lambda h: _hook.__setitem__(0, h)
    mod.get_axon_ntff_profile_hook = lambda: _hook[0]
    import antenv
    sys.modules["antenv.axon_hooks"] = mod
    antenv.axon_hooks = mod
    so = "/opt/axon/libaxon_pjrt.so"
    try:
        hook = _ntff_profile_via_ctypes(so)
    except OSError:
        hook = None
    mod.set_axon_ntff_profile_hook(hook)


# revision 13
# speedup vs baseline: 1.0050x; 1.0050x over previous
"""TRN2 Bass kernel for nn_MultiHeadSelfAttention_15822659518596.

Key algebraic fact: in the reference, softmax and V are dead code — the
output is

    out[b,i,:] = (scores[b,i].reshape(S*H)) @ W_fc.T + b_fc
    scores[b,i,j,n] = (q[b,i,n,:] . k[b,j,n,:]) / 8

which collapses into dense GEMMs without materializing the (B,S,S,H)
score tensor.  With Wq additionally folded PAST the big contraction
(v2 change — the old kernel materialized qT = Wq @ x^T, 536 MMAC/core):

    Kf_b = x_b @ Wk.T                      (S, D)   [c = n*64+kk head-major]
    M_b[c,o]  = sum_j Kf_b[j,c] * Wfc[o, j*8+n(c)] / 8      (D, OH)
    G_b[d,o]  = sum_c Wq[c,d]   * M_b[c,o]                  (D, OH)  <- tiny
    outT[o,i] = sum_d G_b[d,o]  * xT_b[d,i]                 (OH, S)

Sharding: 8 cores = (4 batches) x (2 halves of the fc output dim o).
Each core computes outT[o_half, S] for its (b, h) — no collectives
(on-chip collectives cost 5-20us floors; useless at this kernel size).

v2 (64.2us) vs baseline (75.6us): G-fold (stage-3 536->67 MMAC/core),
dual HWDGE DMA queues, bf16 output, bias instructions compiled only
when biases are nonzero (they are zero here).

v3: PE warm-up matmuls (HAM runs cold 1.2GHz until ~3.4us of sustained
activity), queue rebalance (x first-halves + wfc heads 0-4 on the
scalar queue; a combined 2KB-row wk|wq tensor + x second-halves + wfc
heads 5-7 + output on the sync queue — 1KB-row DMAs measured 173 GB/s
vs 281 for 8KB rows), kf copies on the scalar NX, output assembled to
two 0.5MB 4KB-row DMAs split across queues.
"""

import ml_dtypes
import numpy as np

import concourse.bass as bass
import concourse.tile as tile
from concourse import mybir, bacc
from concourse.bass_utils import run_bass_kernel_spmd

B, S, D, H = 4, 2048, 512, 8
DK = D // H            # 64
OH = D // 2            # 256, per-core o-half
NC = 8                 # cores
F32 = mybir.dt.float32
BF16 = mybir.dt.bfloat16
COPY = mybir.ActivationFunctionType.Identity

_CACHE = {}


def _build_program(with_bias: bool):
    """One SPMD Bass program; per-core tensors differ only in data."""
    nc = bacc.Bacc("TRN2", target_bir_lowering=False, debug=False, num_devices=NC)

    xT = nc.dram_tensor("xT", [D, S], BF16, kind="ExternalInput")          # x_b.T
    # wkq[:, :512] = Wk.T as [d, c]; wkq[:, 512:] = Wq as [c, d].
    # Combined so the sync-queue DMAs have 2KB partition rows.
    wkq = nc.dram_tensor("wkq", [D, 2 * D], BF16, kind="ExternalInput")
    wfc = nc.dram_tensor("wfc", [H // 2, 128, 32 * OH], BF16, kind="ExternalInput")
    if with_bias:
        colsum = nc.dram_tensor("colsum", [1, H * OH], BF16, kind="ExternalInput")
        bkrow = nc.dram_tensor("bkrow", [1, D], BF16, kind="ExternalInput")
        bq_col = nc.dram_tensor("bq_col", [128, 4], BF16, kind="ExternalInput")
        bfc_row = nc.dram_tensor("bfc_row", [1, OH], F32, kind="ExternalInput")
    outT = nc.dram_tensor("outT", [OH, S], BF16, kind="ExternalOutput")

    with tile.TileContext(nc) as tc:
        with tc.tile_pool(name="xt", bufs=4) as p_xt, \
             tc.tile_pool(name="wk", bufs=4) as p_wk, \
             tc.tile_pool(name="kf", bufs=16) as p_kf, \
             tc.tile_pool(name="wf", bufs=8) as p_wf, \
             tc.tile_pool(name="m", bufs=4) as p_m, \
             tc.tile_pool(name="g", bufs=4) as p_g, \
             tc.tile_pool(name="ob", bufs=3) as p_ob, \
             tc.tile_pool(name="bias", bufs=1) as p_bias, \
             tc.tile_pool(name="psA", bufs=(2 if with_bias else 3), space="PSUM") as psA, \
             tc.tile_pool(name="psM", bufs=2, space="PSUM") as psM, \
             tc.tile_pool(name="psG", bufs=2, space="PSUM") as psG:

            # ---- PE warm-up: the HAM clock gate keeps the PE at 1.2GHz
            # until ~3.4us of sustained activity.  Dummy matmuls on a
            # memset scratch tile keep the PE busy (and then warm) while
            # the x/wk DMAs land, so stage 1 runs at 2.4GHz.
            t_wu = p_bias.tile([128, 128], BF16, tag="wu")
            nc.vector.memset(t_wu[:], 0.0)
            pw = psA.tile([128, D], F32, tag="acc")
            for _ in range(96):
                nc.tensor.matmul(pw[:, :128], t_wu[:], t_wu[:],
                                 start=True, stop=True)

            # ---- DMA plan.  Two HWDGE queues (scalar=q10, sync=q1);
            # DMA issues on an engine NX cost ~0.6-1.7us each and >8
            # outstanding DMAs stall on semaphore-lane reuse, so keep
            # counts low and never put latency-critical compute copies
            # on a DMA-issuing engine (kf copies live on gpsimd).
            # scalar: x halves (stage 1 input), then wfc pairs 2,3.
            # sync:   wkq, wfc pairs 0,1, then the output chunks.
            # Arrival tracks stage-2 consumption order u=0..3.
            xts = []
            for di in range(4):
                t_x = p_xt.tile([128, S], BF16, tag="xt")
                nc.scalar.dma_start(t_x[:, :S // 2],
                                    xT[di * 128:(di + 1) * 128, :S // 2])
                xts.append(t_x)
            for di in range(4):
                nc.scalar.dma_start(xts[di][:, S // 2:],
                                    xT[di * 128:(di + 1) * 128, S // 2:])
            wks, wqs, wkqs = [], [], []
            for di in range(4):
                t_kq = p_wk.tile([128, 2 * D], BF16, tag="wk")
                nc.sync.dma_start(t_kq[:], wkq[di * 128:(di + 1) * 128, :])
                wkqs.append(t_kq)
                wks.append(t_kq[:, :D])
                wqs.append(t_kq[:, D:])
            wfs = []
            for u in range(4):
                t_w = p_wf.tile([128, 32 * OH], BF16, tag="wf")
                eng = nc.sync if u < 2 else nc.scalar
                eng.dma_start(t_w[:], wfc[u][:, :])
                wfs.append(t_w[:, :16 * OH])
                wfs.append(t_w[:, 16 * OH:])
            if with_bias:
                t_bk = p_bias.tile([1, D], BF16, tag="bk")
                nc.sync.dma_start(t_bk[:], bkrow[:])
                t_cs = p_bias.tile([1, H * OH], BF16, tag="cs")
                nc.sync.dma_start(t_cs[:], colsum[:])
                t_bq = p_bias.tile([128, 4], BF16, tag="bq")
                nc.sync.dma_start(t_bq[:], bq_col[:])
                t_bfc = p_bias.tile([1, OH], F32, tag="bfc")
                nc.sync.dma_start(t_bfc[:], bfc_row[:])
                t_ones = p_bias.tile([1, 512], BF16, tag="ones")
                nc.vector.memset(t_ones[:], 1.0)

            # ---- stage 1: Kf[j, c] (16 j-tiles), Kf = x @ Wk.T ----
            kfs = []
            for jt in range(16):
                pk = psA.tile([128, D], F32, tag="acc")
                for di in range(4):
                    nc.tensor.matmul(
                        pk[:], xts[di][:, jt * 128:(jt + 1) * 128], wks[di][:],
                        start=(di == 0), stop=(di == 3))
                t_kf = p_kf.tile([128, D], BF16, tag="kf")
                nc.vector.tensor_copy(t_kf[:], pk[:])
                kfs.append(t_kf)

            # ---- stage 2+3 fused per head-pair u: M[c_u, o] then the
            # partial G accumulation G += Wq[c_u,:].T @ M[c_u,:].
            # bf16 matmuls support PE column-group tiling: head 2u
            # accumulates into psum[0:64] (col group 0) while head 2u+1
            # goes to psum[64:128] (col group 64) — one (128, OH) bank.
            pg0 = psG.tile([128, 512], F32, tag="pg")
            pg1 = psG.tile([128, 512], F32, tag="pg")
            pgs = [pg0[:, :OH], pg0[:, OH:], pg1[:, :OH], pg1[:, OH:]]
            # Two G accumulation groups share each bank (512 cols), so a
            # start=True would clear the sibling group's half: memset the
            # banks and run every G matmul start=False instead.
            nc.vector.memset(pg0[:], 0.0)
            nc.vector.memset(pg1[:], 0.0)
            if with_bias:
                pv = psM.tile([1, OH], F32, tag="pv")

            tms = []

            def _emit_g(u):
                if with_bias:
                    # v[o] += bq[c_u] . M[c_u, o]   (rank-1 over the block)
                    nc.tensor.matmul(pv[:], t_bq[:, u:u + 1], tms[u][:],
                                     start=(u == 0), stop=(u == 3))
                for dc in range(4):
                    nc.tensor.matmul(
                        pgs[dc][:], wqs[u][:, dc * 128:(dc + 1) * 128],
                        tms[u][:],
                        start=False, stop=(u == 3 and dc % 2 == 1),
                        skip_group_check=True)

            for u in range(4):
                n0, n1 = 2 * u, 2 * u + 1
                pm = psM.tile([128, OH], F32, tag="pm")
                # Zero the bank with DVE and run every matmul start=False:
                # per-element has_written semantics then make any schedule
                # order of the two disjoint col-group chains correct (a
                # start=True matmul would clear the WHOLE bank and race the
                # other chain, which Tile cannot see as a WAW hazard).
                nc.vector.memset(pm[:], 0.0)
                for jt in range(16):
                    nc.tensor.matmul(
                        pm[0:64, :], kfs[jt][:, n0 * 64:(n0 + 1) * 64],
                        wfs[n0][:, jt * OH:(jt + 1) * OH],
                        start=False, stop=False, tile_position=(0, 0),
                        skip_group_check=True)
                    nc.tensor.matmul(
                        pm[64:128, :], kfs[jt][:, n1 * 64:(n1 + 1) * 64],
                        wfs[n1][:, jt * OH:(jt + 1) * OH],
                        start=False, stop=(not with_bias and jt == 15),
                        tile_position=(0, 64), skip_group_check=True)
                if with_bias:
                    # exact b_qkv k-bias: M += bk[c] (x) colsum_n
                    nc.tensor.matmul(
                        pm[0:64, :], t_bk[0:1, n0 * 64:(n0 + 1) * 64],
                        t_cs[0:1, n0 * OH:(n0 + 1) * OH],
                        start=False, stop=False, tile_position=(0, 0),
                        skip_group_check=True)
                    nc.tensor.matmul(
                        pm[64:128, :], t_bk[0:1, n1 * 64:(n1 + 1) * 64],
                        t_cs[0:1, n1 * OH:(n1 + 1) * OH],
                        start=False, stop=True, tile_position=(0, 64),
                        skip_group_check=True)
                t_m = p_m.tile([128, OH], BF16, tag="m")
                nc.vector.tensor_copy(t_m[:], pm[:])
                tms.append(t_m)
                if u > 0:
                    _emit_g(u - 1)
            _emit_g(3)

            # ---- G psum -> bf16 SBUF; v += b_fc ----
            t_g0 = p_g.tile([128, 512], BF16, tag="g")
            nc.vector.tensor_copy(t_g0[:], pg0[:])
            t_g1 = p_g.tile([128, 512], BF16, tag="g")
            nc.vector.tensor_copy(t_g1[:], pg1[:])
            gs = [t_g0[:, :OH], t_g0[:, OH:], t_g1[:, :OH], t_g1[:, OH:]]
            if with_bias:
                t_vb = p_bias.tile([1, OH], BF16, tag="vb")
                v_f = p_bias.tile([1, OH], F32, tag="vf")
                nc.vector.tensor_add(v_f[:], pv[:], t_bfc[:])
                nc.vector.tensor_copy(t_vb[:], v_f[:])

            # ---- stage 4: outT[o, i] = sum_d G[d,o] * xT[d,i] (+ v[o]) ----
            for oc in range(2):
                for ic in range(4):
                    po = psA.tile([128, 512], F32, tag="acc")
                    if with_bias:
                        nc.tensor.matmul(
                            po[:], t_vb[0:1, oc * 128:(oc + 1) * 128],
                            t_ones[0:1, :], start=True, stop=False)
                    for dc in range(4):
                        nc.tensor.matmul(
                            po[:], gs[dc][:, oc * 128:(oc + 1) * 128],
                            xts[dc][:, ic * 512:(ic + 1) * 512],
                            start=(not with_bias and dc == 0), stop=(dc == 3))
                    t_o = p_ob.tile([128, 512], BF16, tag="ob")
                    nc.vector.tensor_copy(t_o[:], po[:])
                    nc.sync.dma_start(
                        outT[oc * 128:(oc + 1) * 128, ic * 512:(ic + 1) * 512],
                        t_o[:])
    nc.compile()
    return nc


def _prep_inputs(x, W_qkv, b_qkv, W_fc, b_fc):
    """Host-side sharding/layout prep. O(bytes) only — no GEMM work."""
    x = np.ascontiguousarray(x, dtype=np.float32)
    W_qkv = np.asarray(W_qkv, dtype=np.float32)
    b_qkv = np.asarray(b_qkv, dtype=np.float32)
    W_fc = np.asarray(W_fc, dtype=np.float32)
    b_fc = np.asarray(b_fc, dtype=np.float32)
    with_bias = bool(np.any(b_qkv) or np.any(b_fc))

    wq = W_qkv.reshape(H, 3, DK, D)  # [n, {q,k,v}, kk, d]
    wq_cd = wq[:, 0].reshape(D, D)                        # [c, d]
    wkT = wq[:, 1].reshape(D, D).T                        # [d, c]
    wkq = np.ascontiguousarray(
        np.concatenate([wkT, wq_cd], axis=1)).astype(ml_dtypes.bfloat16)
    bq = b_qkv.reshape(H, 3, DK)
    bq_c = np.ascontiguousarray(bq[:, 0].reshape(D))      # c-order
    bk_c = np.ascontiguousarray(bq[:, 1].reshape(D))
    bq_col = np.ascontiguousarray(bq_c.reshape(4, 128).T).astype(ml_dtypes.bfloat16)
    bkrow = bk_c.reshape(1, D).astype(ml_dtypes.bfloat16)

    Wfc_s = W_fc * (1.0 / 8.0)
    # per o-half h: [n, jj, t, o] layout, plus per-head column sums
    wfc_h, cs_h, bfc_h = [], [], []
    for h in range(2):
        A = Wfc_s[h * OH:(h + 1) * OH, :]                  # (256, 16384)
        arr = np.ascontiguousarray(A.T).reshape(S, H, OH).transpose(1, 0, 2)  # [n,j,o]
        cs = np.ascontiguousarray(arr.sum(axis=1)).reshape(1, H * OH)
        arr2 = np.ascontiguousarray(
            arr.reshape(H, 16, 128, OH).transpose(0, 2, 1, 3)  # [n, jj, t, o]
        ).reshape(H, 128, 16 * OH)
        arr2 = np.ascontiguousarray(
            arr2.reshape(4, 2, 128, 16 * OH).transpose(0, 2, 1, 3)
        ).reshape(4, 128, 32 * OH).astype(ml_dtypes.bfloat16)
        wfc_h.append(arr2)
        cs_h.append(cs.astype(ml_dtypes.bfloat16))
        bfc_h.append(np.ascontiguousarray(b_fc[h * OH:(h + 1) * OH].reshape(1, OH)))

    xT_b = [np.ascontiguousarray(x[b].T).astype(ml_dtypes.bfloat16) for b in range(B)]

    in_maps = []
    for c in range(NC):
        b, h = c // 2, c % 2
        m = {
            "xT": xT_b[b],
            "wkq": wkq,
            "wfc": wfc_h[h],
        }
        if with_bias:
            m.update({
                "colsum": cs_h[h],
                "bkrow": bkrow,
                "bq_col": bq_col,
                "bfc_row": bfc_h[h],
            })
        in_maps.append(m)
    return in_maps, with_bias


def _run(in_maps, with_bias, trace=False, **kw):
    key = ("nc", with_bias)
    if key not in _CACHE:
        _CACHE[key] = _build_program(with_bias)
    return run_bass_kernel_spmd(
        _CACHE[key], in_maps, core_ids=list(range(NC)), trace=trace, **kw)


def _assemble(results):
    out = np.empty((B, S, D), dtype=np.float32)
    for c in range(NC):
        b, h = c // 2, c % 2
        out[b, :, h * OH:(h + 1) * OH] = results[c]["outT"].T.astype(np.float32)
    return out


def kernel(x, W_qkv, b_qkv, W_fc, b_fc):
    in_maps, with_bias = _prep_inputs(x, W_qkv, b_qkv, W_fc, b_fc)
    res = _run(in_maps, with_bias, trace=False)
    return _assemble(res.results)


def kernel_traced(x, W_qkv, b_qkv, W_fc, b_fc):
    """Like kernel() but returns (out, BassKernelResults) with NTFF trace."""
    import os
    os.environ.setdefault("BASS_PERFETTO_PROFILE_ALL_CORES", "1")
    _install_ntff_hook_shim()
    in_maps, with_bias = _prep_inputs(x, W_qkv, b_qkv, W_fc, b_fc)
    res = _run(in_maps, with_bias, trace=True)
    return _assemble(res.results), res


def _install_ntff_hook_shim():
    """The agent image's antenv lacks axon_hooks; provide it so
    run_bass_kernel_spmd(trace=True) can reach the NTFF profiler."""
    import sys, types
    if "antenv.axon_hooks" in sys.modules:
        return
    try:
        from trn_agent_boot.trn_boot import _ntff_profile_via_ctypes
    except ImportError:
        return
    mod = types.ModuleType("antenv.axon_hooks")
    _hook = [None]
    mod.set_axon_ntff_profile_hook = lambda h: _hook.__setitem__(0, h)
    mod.get_axon_ntff_profile_hook = lambda: _hook[0]
    import antenv
    sys.modules["antenv.axon_hooks"] = mod
    antenv.axon_hooks = mod
    so = "/opt/axon/libaxon_pjrt.so"
    try:
        hook = _ntff_profile_via_ctypes(so)
    except OSError:
        hook = None
    mod.set_axon_ntff_profile_hook(hook)
